# revision 49
# baseline (speedup 1.0000x reference)
"""DIN-attention kernel for Trainium2, 8-core SPMD.

Reference computation (per batch b, seq pos l, x = item_seq[b, l]):
    mlp_in = [tgt, x, x-tgt, x*tgt]           (4D = 512)
    h      = relu(mlp_in @ W1 + b1)           (2D = 256)
    score  = h @ W2 + b2                      (1)
    out_b  = sum_l score[l] * x[l] * (l < seq_len[b])

Algebraic restructure (W1 = [A; B; C; Dm] in 128-row blocks):
    z   = x @ Wx + y @ Wy + c_b,   Wx = B + C, Wy = Dm, y = x * tgt_b,
    c_b = tgt_b @ (A - C) + b1
    out = sum_{l < n_b} (W2.T relu(z) + b2) * x[l]

Device strategy (per core):
  - Batches sorted by seq_len descending; slot s holds global ranks
    [8s, 8s+8), one per core, padded to a shared per-slot length; slots
    interleaved long/short so per-tile reduce counts stay uniform.
  - ONE fp8 quint stream per token (x8, rx, y8, ry16, ind) = 5 B/row:
    x8 = fp8(x), rx = fp8(x - x8), ry16 = fp8(16(y - y8)), plus the
    one-hot bias indicator.  Per hidden half, PSUM gets
      16 z = (16W8x, 16W8x).(x8, rx) + (16W8y, W8y).(y8, ry16)
           + (E16x, C8).(x8, ind) + (E16y, CR).(y8, ind)
    4 DoubleRow matmuls at 0.5 cyc/col; relu applies scale 1/16.
  - No separate bf16 x stream: the reduce-side xs = x8 + rx is
    rebuilt on the Pool engine (gpsimd Add, bf16 out, per chunk).
  - Engine assignment (GPSIMD cannot touch PSUM on TRN2 hardware):
    ACT does every relu (one op per tile covering both hidden halves via
    2-block strided APs over a contiguous 2-bank z PSUM tile), DVE does
    every per-slot fused reduce ((score + b2) * xs, accum into
    acc[:, s]), and Pool rebuilds xs -- each engine lands below the
    Tensor engine's ~66 us of matmul work.
  - Software pipelining: tile i's score matmuls + reduces are emitted
    two tiles later, giving each relu two tiles of slack; dummy warm-up
    matmuls during the DMA fill ramp the PE clock; chunk DMAs are
    prefetched 3-4 ahead through a 5-buffer ring; the 16 longest slot
    groups sit un-interleaved at the stream end so the post-PE drain is
    short, and most of the output is DMA'd out early.
"""

import sys

import numpy as np

for _p in ("/opt/trn_rl_repo",):
    if _p not in sys.path:
        sys.path.insert(0, _p)

import concourse.bacc as bacc
import concourse.bass as bass
import concourse.tile as tile
from concourse import mybir
from concourse.alu_op_type import AluOpType
from concourse.bass_utils import run_bass_kernel_spmd

assert bass  # re-exported for callers

B_FULL = 2048
L_FULL = 200
D = 128
N_CORES = 8
HID = 256  # 2D
TILE_N = 512  # fp32 PSUM bank columns
F32 = mybir.dt.float32
F32R = mybir.dt.float32r
BF16 = mybir.dt.bfloat16
FP8 = mybir.dt.float8e4
DRMODE = mybir.MatmulPerfMode.DoubleRow

WIN = 64  # slots per bias window (one-hot padded to 128 partitions)
B2VAL = [0.0]  # b2 constant, set by build_all before tracing

# cost-model rates (ns) used for greedy engine load balancing
ACT_NS = 0.833
ACT_OP = 185.0
DVE_NS = 1.042
DVE_OP_PSUM = 125.0
DVE_OP_SBUF = 60.0
POOL_NS = 1.389
POOL_OP = 95.0

# feature flags (A/B tuning; flip from profilers)
CFG = {
    "red_split": True,    # reduces greedily DVE vs Pool (False: DVE only)
    "xs_per_tile": False,  # chunk-level xs: fewer Pool launches
}


def _plan(seq_len):
    """Slot / tile / chunk plan shared by all cores (SPMD)."""
    n = np.clip(np.asarray(seq_len).astype(np.int64), 0, L_FULL)
    order = np.argsort(-n, kind="stable")  # descending
    n_sorted = n[order]
    rank_lens = []
    for s in range(B_FULL // N_CORES):
        m = int(n_sorted[N_CORES * s])  # max of ranks [8s, 8s+8)
        if m <= 0:
            break
        rank_lens.append(m)  # odd ok: bf16 score matmul has no column restriction
    S = len(rank_lens)
    # Interleave long and short slots in the body so the per-tile count of
    # reduce ops (one per slot) stays uniform; the K longest rank-groups go
    # UN-interleaved at the very end, so the final tiles hold the fewest
    # slots and the post-PE reduce tail is as short as possible.
    tail_k = min(16, S)
    perm = []
    lo, hi = tail_k, S - 1
    while lo <= hi:
        perm.append(lo)
        if hi != lo:
            perm.append(hi)
        lo += 1
        hi -= 1
    perm.extend(range(tail_k - 1, -1, -1))  # ..., rank 1, rank 0 (longest last)
    perm = np.asarray(perm, dtype=np.int64)  # stream slot j holds rank-group perm[j]
    slot_lens = [rank_lens[int(p)] for p in perm]
    offs = np.zeros(S + 1, dtype=np.int64)
    offs[1:] = np.cumsum(slot_lens)
    T = int(offs[-1])

    # Slot-aligned tiles: whole slots, <= TILE_N tokens, never crossing a
    # WIN-slot window boundary.
    tiles = []  # (slot_a, slot_b)  [a, b) slots
    sa = 0
    while sa < S:
        sb = sa
        wend = (sa // WIN + 1) * WIN
        while (
            sb < S
            and sb < wend
            and offs[sb + 1] - offs[sa] <= TILE_N
        ):
            sb += 1
        if sb == sa:
            sb = sa + 1
        tiles.append((sa, sb))
        sa = sb

    # Chunks: groups of whole tiles with tapered token budgets.  Small first
    # chunks start compute early; steady-state 1792 keeps the DMA well ahead
    # through a 5-deep prefetch ring.
    budgets = [512, 1024, 1024, 1280]
    chunks = []  # (tile_a, tile_b, tok_off, tok_len)
    ta = 0
    bi = 0
    while ta < len(tiles):
        if bi < len(budgets):
            cap = budgets[bi]
        else:
            cap = 1792
        bi += 1
        tb = ta
        start = int(offs[tiles[ta][0]])
        while tb < len(tiles) and int(offs[tiles[tb][1]]) - start <= cap:
            tb += 1
        if tb == ta:
            tb = ta + 1
        end = int(offs[tiles[tb - 1][1]])
        chunks.append((ta, tb, start, end - start))
        ta = tb
    return n, order, perm, slot_lens, offs, T, tiles, chunks


def _build_program(slot_lens, offs, T, tiles, chunks):
    S = len(slot_lens)
    NW = (S + WIN - 1) // WIN
    nc = bacc.Bacc("TRN2", target_bir_lowering=False, debug=False)

    xq_d = nc.dram_tensor("xq", [D, 5 * T], FP8, kind="ExternalInput")
    WQ2 = 4 * 2 * D + NW * 4 * 2 * D  # m1/m2 blocks + (w,h) pair blocks
    wq_d = nc.dram_tensor("wq", [D, WQ2], FP8, kind="ExternalInput")
    w2r_d = nc.dram_tensor("w2r", [D, HID], BF16, kind="ExternalInput")
    out_d = nc.dram_tensor("out_t", [D, 256], F32, kind="ExternalOutput")

    cmax = max(c[3] for c in chunks)
    n_tiles = len(tiles)

    # greedy engine-load balance state (ns)
    load = {"act": 0.0, "dve": 0.0, "pool": 0.0}

    with tile.TileContext(nc) as tc:
        with (
            tc.tile_pool(name="const", bufs=1) as cpool,
            tc.tile_pool(name="xst", bufs=6) as xpool,
            tc.tile_pool(name="sst", bufs=3) as spool,
            tc.tile_pool(name="rst", bufs=5) as rpool,
            tc.tile_pool(name="dst", bufs=4) as dpool,
            tc.tile_pool(name="zps", bufs=2, space="PSUM") as zpool,
            tc.tile_pool(name="pps", bufs=4, space="PSUM") as ppool,
        ):
            wq = cpool.tile([D, WQ2], FP8, tag="wq")
            w2r = cpool.tile([D, HID], BF16, tag="w2r")
            acc = cpool.tile([D, 256], F32, tag="acc")

            nc.vector.memset(acc[:], 0.0)
            first = True
            out_sent = [0]  # acc columns already DMA'd out
            pending = []  # deque of (sa, sb, c0, toff, r, xs, nn)

            def flush_one():
                if not pending:
                    return
                psa, psb, pc0, ptoff, pr, pxs, pnn = pending.pop(0)
                pbc = ppool.tile([D, TILE_N], F32, tag="pbc")
                nc.tensor.matmul(
                    pbc[:, :pnn], w2r[:, 0:D], pr[:, :pnn],
                    start=True, stop=False,
                )
                nc.tensor.matmul(
                    pbc[:, :pnn], w2r[:, D:HID], pr[:, TILE_N : TILE_N + pnn],
                    start=False, stop=True,
                )
                # NOTE: GPSIMD cannot access PSUM on TRN2 hardware, so every
                # reduce (reads the PSUM score) must run on DVE.
                dumpd = dpool.tile([D, TILE_N], F32, tag="dumpd")
                for s in range(psa, psb):
                    a = int(offs[s] - ptoff)
                    b = int(offs[s + 1] - ptoff)
                    ln = b - a
                    eng, dump = nc.vector, dumpd
                    load["dve"] += DVE_NS * ln + DVE_OP_PSUM
                    eng.scalar_tensor_tensor(
                        out=dump[:, a - pc0 : b - pc0],
                        in0=pbc[:, a - pc0 : b - pc0],
                        scalar=B2VAL[0],
                        in1=pxs[:, a:b],
                        op0=AluOpType.add,
                        op1=AluOpType.mult,
                        accum_out=acc[:, s : s + 1],
                    )
                # overlap most of the output writeback with the tail tiles
                if psb >= (S * 3) // 4 and psb - out_sent[0] >= 32 and psb < S:
                    a0 = out_sent[0]
                    out_sent[0] = int(psb)
                    nc.sync.dma_start(
                        out=out_d[:, a0 : out_sent[0]], in_=acc[:, a0 : out_sent[0]]
                    )

            xq_tiles = {}

            def fetch_chunk(ci):
                if ci in xq_tiles or ci >= len(chunks):
                    return
                _, _, toff, tlen = chunks[ci]
                xqt = xpool.tile([D, 5 * cmax], FP8, tag="xq")
                nc.sync.dma_start(
                    out=xqt[:, : 5 * tlen],
                    in_=xq_d[:, 5 * toff : 5 * toff + 5 * tlen],
                )
                xq_tiles[ci] = xqt

            for ci, (ta, tb, toff, tlen) in enumerate(chunks):
                if first:
                    # p-state warm-up: dummy DoubleRow matmuls on a zeroed
                    # scratch tile keep PE continuously busy from ~0.5us, so
                    # real tiles start at (near) full clock
                    wscr = cpool.tile([D, 2 * D], FP8, tag="wscr")
                    nc.vector.memset(wscr[:], 0.0)
                    # wq head: m1/m2 + window-0 bias blocks feed the first tiles
                    nc.sync.dma_start(out=wq[:, : 16 * D], in_=wq_d[:, : 16 * D])
                    fetch_chunk(0)
                    fetch_chunk(1)
                    wst = wscr[:].rearrange("p (t m) -> p t m", t=2)
                    wmov = wscr[:].rearrange("p (n q) -> p q n", q=2)
                    for wi in range(26):
                        wz = ppool.tile([D, TILE_N], F32, tag="pbc")
                        nc.tensor.matmul(
                            wz[:, :D], wst, wmov,
                            start=True, stop=True, perf_mode=DRMODE,
                        )
                    nc.sync.dma_start(out=w2r[:], in_=w2r_d[:])
                    fetch_chunk(2)
                    fetch_chunk(3)
                    first = False
                # prefetch chunks ahead of consumption (ring depth 5)
                for k in range(ci + 1, min(ci + 5, len(chunks))):
                    fetch_chunk(k)
                if ci == 1 and WQ2 > 16 * D:
                    # windows 1+ bias blocks: first needed ~slot 64 (~chunk 5)
                    nc.sync.dma_start(out=wq[:, 16 * D :], in_=wq_d[:, 16 * D :])
                xqt = xq_tiles.pop(ci)
                xs = spool.tile([D, cmax], BF16, tag="xs")
                qv = xqt[:].rearrange("p (n q) -> p q n", q=5)
                if not CFG["xs_per_tile"]:
                    # xs = x8 + rx (bf16), one Pool Add per chunk
                    nc.gpsimd.tensor_add(
                        out=xs[:, :tlen],
                        in0=qv[:, 0, 0:tlen],
                        in1=qv[:, 1, 0:tlen],
                    )
                    load["pool"] += POOL_NS * tlen / 0.7 + POOL_OP

                for ti in range(ta, tb):
                    sa, sb = tiles[ti]
                    c0 = int(offs[sa] - toff)
                    c1 = int(offs[sb] - toff)
                    nn = c1 - c0
                    w = sa // WIN  # single window per tile by construction
                    if CFG["xs_per_tile"]:
                        # xs = x8 + rx on Pool (gpsimd "Add" ISA op) -- the
                        # only PSUM-free elementwise work on this engine
                        nc.gpsimd.tensor_add(
                            out=xs[:, c0:c1],
                            in0=qv[:, 0, c0:c1],
                            in1=qv[:, 1, c0:c1],
                        )
                        load["pool"] += POOL_NS * nn / 0.7 + POOL_OP

                    z = zpool.tile([D, 2 * TILE_N], F32, tag="z")
                    for h in (0, 1):

                        def wp(off):
                            return wq[:, off : off + 2 * D].rearrange(
                                "p (t m) -> p t m", t=2
                            )

                        zh = z[:, h * TILE_N : h * TILE_N + nn]
                        pb = 8 * D + (w * 2 + h) * 4 * D
                        nc.tensor.matmul(
                            zh, wp(h * 4 * D), qv[:, 0:2, c0:c1],
                            start=True, stop=False, perf_mode=DRMODE,
                        )
                        nc.tensor.matmul(
                            zh, wp(h * 4 * D + 2 * D), qv[:, 2:4, c0:c1],
                            start=False, stop=False, perf_mode=DRMODE,
                        )
                        # (E16x | C8).(x8, ind) and (E16y | CR).(y8, ind)
                        nc.tensor.matmul(
                            zh, wp(pb), qv[:, 0:5:4, c0:c1],
                            start=False, stop=False, perf_mode=DRMODE,
                        )
                        nc.tensor.matmul(
                            zh, wp(pb + 2 * D), qv[:, 2:5:2, c0:c1],
                            start=False, stop=True, perf_mode=DRMODE,
                        )

                    r = rpool.tile([D, 2 * TILE_N], BF16, tag="r")
                    # ONE relu op covering both hidden halves via 2-block APs
                    zv = z[:].rearrange("p (b n) -> p b n", b=2)[:, :, :nn]
                    rv = r[:].rearrange("p (b n) -> p b n", b=2)[:, :, :nn]
                    nc.scalar.activation(
                        rv, zv,
                        mybir.ActivationFunctionType.Relu, scale=1.0 / 16.0,
                    )
                    load["act"] += ACT_NS * 2 * nn + ACT_OP
                    # score + reduce of the tile TWO back, so each relu gets
                    # two full tiles of slack (deep software pipelining);
                    # drain eagerly near the end to shorten the reduce tail
                    pending.append((sa, sb, c0, toff, r, xs, nn))
                    depth = 2 if ti < n_tiles - 2 else 1
                    while len(pending) > depth:
                        flush_one()

            while pending:
                flush_one()
            nc.sync.dma_start(
                out=out_d[:, out_sent[0] :], in_=acc[:, out_sent[0] :]
            )
    nc.compile()
    return nc


def _pack_core(item_seq, target, nvec, order, perm, slot_lens, offs, T, core):
    from ml_dtypes import float8_e4m3

    S = len(slot_lens)
    x_nat = np.zeros((T, D), dtype=np.float32)
    y_nat = np.zeros((T, D), dtype=np.float32)

    ind = np.zeros((D, T), dtype=float8_e4m3)
    for s in range(S):
        b = int(order[N_CORES * int(perm[s]) + core])
        o = int(offs[s])
        nb = int(nvec[b])
        if nb > 0:
            x_nat[o : o + nb] = item_seq[b, :nb]
            y_nat[o : o + nb] = item_seq[b, :nb] * target[b]
        ind[s % WIN, o : o + slot_lens[s]] = 1.0
    xT = np.ascontiguousarray(x_nat.T)
    yT = np.ascontiguousarray(y_nat.T)
    x8 = xT.astype(float8_e4m3)
    y8 = yT.astype(float8_e4m3)
    rx = (xT - x8.astype(np.float32)).astype(float8_e4m3)
    ry = (16.0 * (yT - y8.astype(np.float32))).astype(float8_e4m3)
    xq = np.empty((D, 5 * T), dtype=float8_e4m3)
    xq[:, 0::5], xq[:, 1::5], xq[:, 2::5] = x8, rx, y8
    xq[:, 3::5], xq[:, 4::5] = ry, ind
    return {"xq": xq}


def build_all(target, item_seq, seq_len, W1, b1, W2, b2):
    """Build (nc, in_maps, assemble) without running — used by kernel()
    and by test harnesses that want to run/profile the program."""
    target = np.asarray(target, dtype=np.float32)
    item_seq = np.asarray(item_seq, dtype=np.float32)
    W1 = np.asarray(W1, dtype=np.float32)
    b1 = np.asarray(b1, dtype=np.float32)
    W2 = np.asarray(W2, dtype=np.float32)
    b2 = np.asarray(b2, dtype=np.float32)

    nvec, order, perm, slot_lens, offs, T, tiles, chunks = _plan(seq_len)

    from ml_dtypes import float8_e4m3

    def f8(a):
        return np.asarray(a, dtype=np.float32).astype(float8_e4m3)

    W1a, W1b = W1[0:D], W1[D : 2 * D]
    W1c, W1d = W1[2 * D : 3 * D], W1[3 * D : 4 * D]
    wbc = np.ascontiguousarray(W1b + W1c)  # x-side weights (128, 256)
    wd = np.ascontiguousarray(W1d)  # y-side weights
    w8x, w8y = f8(wbc), f8(wd)
    w16x = f8(16.0 * w8x.astype(np.float32))
    w16y = f8(16.0 * w8y.astype(np.float32))
    assert np.array_equal(w16x.astype(np.float32), 16.0 * w8x.astype(np.float32))
    assert np.array_equal(w16y.astype(np.float32), 16.0 * w8y.astype(np.float32))
    ex16 = f8(16.0 * (wbc - w8x.astype(np.float32)))
    ey16 = f8(16.0 * (wd - w8y.astype(np.float32)))
    # Per half h: DR pairs (16W8x, W8x).(x8, rx16) + (16W8y, W8y).(y8, ry16)
    gblocks = np.empty((D, 8 * D), dtype=float8_e4m3)
    for h in (0, 1):
        hs = slice(h * D, h * D + D)
        base = h * 4 * D
        gblocks[:, base + 0 * D : base + 1 * D] = w16x[:, hs]
        gblocks[:, base + 1 * D : base + 2 * D] = w16x[:, hs]
        gblocks[:, base + 2 * D : base + 3 * D] = w16y[:, hs]
        gblocks[:, base + 3 * D : base + 4 * D] = w8y[:, hs]
    cmat = (16.0 * (target @ (W1a - W1c) + b1)).astype(np.float32)  # (B, 256)
    from ml_dtypes import bfloat16
    w2r = np.empty((D, HID), dtype=bfloat16)
    w2r[:, 0:D] = np.repeat(W2[0:D, 0:1], D, axis=1)  # [k, m] = W2[k]
    w2r[:, D:HID] = np.repeat(W2[D:HID, 0:1], D, axis=1)
    B2VAL[0] = float(np.asarray(b2).reshape(-1)[0])

    nc = _build_program(slot_lens, offs, T, tiles, chunks)

    S = len(slot_lens)
    NW = (S + WIN - 1) // WIN
    shared = {"w2r": w2r}
    in_maps = []
    for k in range(N_CORES):
        m = _pack_core(item_seq, target, nvec, order, perm, slot_lens, offs, T, k)
        m.update(shared)
        cstack = np.zeros((NW, D, HID), dtype=np.float32)  # slot-rows x hidden
        for s in range(S):
            b = int(order[N_CORES * int(perm[s]) + k])
            cstack[s // WIN, s % WIN, :] = cmat[b]
        wqc = np.zeros((D, 8 * D + NW * 8 * D), dtype=float8_e4m3)
        wqc[:, : 8 * D] = gblocks
        for w_ in range(NW):
            for h in (0, 1):
                cw = cstack[w_][:, h * D : h * D + D]
                c8 = cw.astype(float8_e4m3)
                cr = (cw - c8.astype(np.float32)).astype(float8_e4m3)
                pb = 8 * D + (w_ * 2 + h) * 4 * D
                wqc[:, pb + 0 * D : pb + 1 * D] = ex16[:, h * D : h * D + D]
                wqc[:, pb + 1 * D : pb + 2 * D] = c8
                wqc[:, pb + 2 * D : pb + 3 * D] = ey16[:, h * D : h * D + D]
                wqc[:, pb + 3 * D : pb + 4 * D] = cr
        m["wq"] = wqc
        in_maps.append(m)

    def assemble(results):
        out = np.zeros((B_FULL, D), dtype=np.float32)
        for k in range(N_CORES):
            ot = np.asarray(results[k]["out_t"])  # (128, 256)
            for s in range(S):
                out[int(order[N_CORES * int(perm[s]) + k])] = ot[:, s]
        return out

    return nc, in_maps, assemble


def kernel(target, item_seq, seq_len, W1, b1, W2, b2):
    nc, in_maps, assemble = build_all(target, item_seq, seq_len, W1, b1, W2, b2)
    res = run_bass_kernel_spmd(nc, in_maps, list(range(N_CORES)))
    results = res.results if hasattr(res, "results") else res
    return assemble(results)
